# revision 1
# baseline (speedup 1.0000x reference)
"""Trainium2 Bass kernel for per-sample 2-expert MoE residual MLP.

Reference computation (per sample b, expert e = cond[b]):
    h = relu(Wd[e] @ x_b + bd[e])        # [MID, H*W]
    y = Wu[e] @ h + bu[e] + x_b          # [C, H*W]

Shapes: x [8, 1024, 64, 64] f32, Wd [2, 256, 1024], bd [2, 256],
        Wu [2, 1024, 256], bu [2, 1024], cond [8] int.

Sharding: data-parallel over batch — one sample per NeuronCore (8 cores).
The expert gather (Wd[cond[b]]) happens on host while building each
core's input map.

Per-core schedule: PASS_N passes over spatial column stripes.
  sync ring   : x stripe in (fp32, 4KB-contiguous rows)
  gpsimd      : xb = bf16(x)   then   x += bu (per-channel, in place)
                -> the y epilogue needs only ONE DVE add: y = psum + x'
  PE          : GEMM1 (bf16, fp32 PSUM, weights loaded once per (m,k)),
                GEMM2 likewise
  scalar (ACT): bias+ReLU+bf16-cast of h from PSUM; issues y-out DMAs
  vector (DVE): y = psum + x' from PSUM to SBUF
  scalar ring : y stripe out
Residual path stays fp32 end-to-end; only GEMM multiplicands are bf16.
"""

import numpy as np
import ml_dtypes
from contextlib import ExitStack

import concourse.bacc as bacc
import concourse.mybir as mybir
import concourse.tile as tile
from concourse.bass_utils import run_bass_kernel_spmd

# Problem dims (hardcoded per contract).
B = 8
C = 1024
MID = 256
H = 64
W = 64
HW = H * W  # 4096

P = 128              # partitions
NB = 512             # matmul free dim / one fp32 PSUM bank
PASS_W = 1024        # spatial columns per pass
NBP = PASS_W // NB   # psum tiles per stripe
PASS_N = HW // PASS_W
KC = C // P          # 8  k-tiles for GEMM1 / m-tiles for GEMM2
KM = MID // P        # 2  m-tiles for GEMM1 / k-tiles for GEMM2

F32 = mybir.dt.float32
BF16 = mybir.dt.bfloat16


def build_nc():
    """Build the per-core Bass program (SPMD: same program on all cores)."""
    nc = bacc.Bacc("TRN2", target_bir_lowering=False, debug=False)

    x_d = nc.dram_tensor("x", [C, HW], F32, kind="ExternalInput")
    # Host pre-tiles the weights to [P, ...] so each partition's row is one
    # contiguous 4KB chunk (fast DMA descriptors, single transfer each).
    wdT_d = nc.dram_tensor("wdT", [P, KC, MID], BF16, kind="ExternalInput")
    wuT_d = nc.dram_tensor("wuT", [P, KM, C], BF16, kind="ExternalInput")
    bd_d = nc.dram_tensor("bd", [P, KM], F32, kind="ExternalInput")
    bu_d = nc.dram_tensor("bu", [P, KC], F32, kind="ExternalInput")
    y_d = nc.dram_tensor("y", [C, HW], F32, kind="ExternalOutput")

    with tile.TileContext(nc) as tc, ExitStack() as ctx:
        wpool = ctx.enter_context(tc.tile_pool(name="w", bufs=1))
        xpool = ctx.enter_context(tc.tile_pool(name="xp", bufs=3))
        xbpool = ctx.enter_context(tc.tile_pool(name="xbp", bufs=2))
        hpool = ctx.enter_context(tc.tile_pool(name="hp", bufs=2))
        ypool = ctx.enter_context(tc.tile_pool(name="yp", bufs=6))
        psh = ctx.enter_context(tc.tile_pool(name="ph", bufs=2, space="PSUM"))
        psy = ctx.enter_context(tc.tile_pool(name="py", bufs=2, space="PSUM"))

        # Resident weights and biases. Scalar HWDGE ring: it is idle at t=0
        # (y-outs start much later) and far faster than gpsimd SWDGE, so the
        # first GEMM1 isn't stalled on weights.
        wd_s = wpool.tile([P, KC, MID], BF16, tag="wd")
        nc.scalar.dma_start(wd_s[:], wdT_d[:])
        wu_s = wpool.tile([P, KM, C], BF16, tag="wu")
        nc.scalar.dma_start(wu_s[:], wuT_d[:])
        bd_s = wpool.tile([P, KM], F32, tag="bd")
        nc.scalar.dma_start(bd_s[:], bd_d[:])
        bu_s = wpool.tile([P, KC], F32, tag="bu")
        nc.scalar.dma_start(bu_s[:], bu_d[:])

        def emit_load(p):
            """x stripe DMA-in (sync ring) + bf16 cast (DVE)."""
            c0 = p * PASS_W
            xt = xpool.tile([P, KC, PASS_W], F32, tag="xt", name=f"xt{p}")
            # Pass 0 loads in half-stripes so GEMM1 can start sooner.
            splits = 2 if p == 0 else 1
            sw = PASS_W // splits
            for sp in range(splits):
                for k in range(KC):
                    nc.sync.dma_start(
                        xt[:, k, sp * sw:(sp + 1) * sw],
                        x_d[k * P:(k + 1) * P, c0 + sp * sw:c0 + (sp + 1) * sw],
                    )
            # bf16 copy for GEMM1 (DVE; gpsimd is ~7x too slow for this).
            xb = xbpool.tile([P, KC, PASS_W], BF16, tag="xb", name=f"xb{p}")
            for sp in range(splits):
                for k in range(KC):
                    nc.vector.tensor_copy(
                        xb[:, k, sp * sw:(sp + 1) * sw],
                        xt[:, k, sp * sw:(sp + 1) * sw],
                    )
            return xt, xb

        loaded = emit_load(0)
        for p in range(PASS_N):
            c0 = p * PASS_W
            xt, xb = loaded

            # GEMM1: h[m] = relu(sum_k wd[k,m].T @ x[k] + bd[m]) -> bf16
            ht = hpool.tile([P, KM, PASS_W], BF16, tag="ht")
            for m in range(KM):
                ph = psh.tile([P, NBP, NB], F32, tag="ph")
                for k in range(KC):
                    for nb in range(NBP):
                        nc.tensor.matmul(
                            ph[:, nb, :],
                            wd_s[:, k, m * P:(m + 1) * P],
                            xb[:, k, nb * NB:(nb + 1) * NB],
                            start=(k == 0),
                            stop=(k == KC - 1),
                        )
                nc.scalar.activation(
                    ht[:, m, :], ph[:],
                    mybir.ActivationFunctionType.Relu,
                    bias=bd_s[:, m:m + 1],
                )

            # GEMM2 + residual: y[mc] = sum_km wu[km,mc].T @ h[km] + bu + x[mc]
            for mc in range(KC):
                # Prefetch next stripe mid-GEMM2: x DMAs + casts land between
                # this stripe's first and second half of residual adds on the
                # in-order DVE stream, so early y tiles drain promptly while
                # casts still precede the next GEMM1.
                if mc == KC // 2 and p + 1 < PASS_N:
                    loaded = emit_load(p + 1)
                py = psy.tile([P, NBP, NB], F32, tag="py")
                for km in range(KM):
                    for nb in range(NBP):
                        nc.tensor.matmul(
                            py[:, nb, :],
                            wu_s[:, km, mc * P:(mc + 1) * P],
                            ht[:, km, nb * NB:(nb + 1) * NB],
                            start=(km == 0),
                            stop=(km == KM - 1),
                        )
                yt = ypool.tile([P, PASS_W], F32, tag="yt")
                # Whole epilogue in one DVE op: yt = (py + bu) + x
                nc.vector.scalar_tensor_tensor(
                    yt[:], py[:], bu_s[:, mc:mc + 1], xt[:, mc, :],
                    mybir.AluOpType.add, mybir.AluOpType.add,
                )
                # y-out alternates between the scalar HWDGE ring and gpsimd's
                # SWDGE queue: two independent DMA queues, and neither ACT nor
                # the Q7 pays the full issue cost (SWDGE issue is ~1.4us/DMA,
                # which alone would serialize the kernel tail).
                if mc % 2 == 0:
                    nc.scalar.dma_start(y_d[mc * P:(mc + 1) * P, c0:c0 + PASS_W], yt[:])
                else:
                    nc.gpsimd.dma_start(y_d[mc * P:(mc + 1) * P, c0:c0 + PASS_W], yt[:])

    nc.compile()
    return nc


_NC = None


def get_nc():
    global _NC
    if _NC is None:
        _NC = build_nc()
    return _NC


def make_in_maps(inputs):
    x = np.asarray(inputs["x"], dtype=np.float32)
    Wd = np.asarray(inputs["Wd"], dtype=np.float32)
    bd = np.asarray(inputs["bd"], dtype=np.float32)
    Wu = np.asarray(inputs["Wu"], dtype=np.float32)
    bu = np.asarray(inputs["bu"], dtype=np.float32)
    cond = np.asarray(inputs["cond"]).astype(np.int64)

    in_maps = []
    for b in range(B):
        e = int(cond[b])
        in_maps.append({
            "x": np.ascontiguousarray(x[b].reshape(C, HW)),
            # [C, MID] -> [KC, P, MID] -> [P, KC, MID] partition-major tiling
            "wdT": np.ascontiguousarray(
                Wd[e].T.reshape(KC, P, MID).transpose(1, 0, 2)
            ).astype(ml_dtypes.bfloat16),
            # [MID, C] -> [KM, P, C] -> [P, KM, C]
            "wuT": np.ascontiguousarray(
                Wu[e].T.reshape(KM, P, C).transpose(1, 0, 2)
            ).astype(ml_dtypes.bfloat16),
            "bd": np.ascontiguousarray(bd[e].reshape(KM, P).T),  # [P, KM]
            "bu": np.ascontiguousarray(bu[e].reshape(KC, P).T),  # [P, KC]
        })
    return in_maps


def run_sharded(inputs, **kwargs):
    """Run on all 8 cores; returns (stacked output [B,C,H,W], BassKernelResults)."""
    nc = get_nc()
    in_maps = make_in_maps(inputs)
    res = run_bass_kernel_spmd(nc, in_maps, core_ids=list(range(B)), **kwargs)
    out = np.stack([res.results[b]["y"].reshape(C, H, W) for b in range(B)])
    return out, res


def kernel(**inputs) -> np.ndarray:
    out, _ = run_sharded(inputs)
    return out

